# revision 7
# baseline (speedup 1.0000x reference)
"""COLoRA linear kernel for 8 Trainium2 NeuronCores.

Reference computation (per batch element b with task t = task_ids[b]):

    out[b] = x[b] @ W.T + bias
           + cw      * 2 * (x[b] @ shared_A.T)    @ shared_B.T
           + (1-cw)  * 2 * (x[b] @ expert_A[t].T) @ expert_B[t].T
    cw = sigmoid(collab_w)

The rank-8 adapters fold exactly into the dense weight (associativity):

    W_eff[b] = W + cw*2*(shared_B @ shared_A) + (1-cw)*2*(expert_B[t] @ expert_A[t])
    out[b]   = x[b] @ W_eff[b].T + bias

so the device kernel is a single memory-bound GEMM per core. Sharding is
data-parallel over batch: core c handles batch element c (B == n_cores == 8).
The MoE routing (task_ids gather) happens on the host at dispatch time.

x is pre-transposed on the host to [d_in, s] so that the contraction dim
lands on SBUF partitions with fully contiguous DMA access patterns — no
on-chip transpose is needed.  Matmuls run in float32r (fp32 storage,
single-pass reduced-precision PE mode: 1 cycle/row at moving dim >= 256).
"""

import os

import numpy as np

import concourse.bass as bass
import concourse.tile as tile
from concourse import bacc, mybir
from concourse.bass_utils import run_bass_kernel_spmd

try:  # tracing (BASS_TRACE) needs the axon NTFF hook; scrub if unavailable
    from antenv.axon_hooks import get_axon_ntff_profile_hook  # noqa: F401
except ImportError:
    os.environ.pop("BASS_TRACE", None)

N_CORES = 8
S = 4096        # rows per core (sequence length; one batch element per core)
D_IN = 1024
D_OUT = 1024
KC = D_IN // 128   # contraction chunks of 128
S_MACRO = 512      # s rows loaded per x DMA
N_HALF = 512       # psum free dim (one bank)
SCALING = 2.0      # lora alpha/r = 16/8

# bf16 operands: single-pass PE matmuls with FWL weight loads, half the
# input DMA bytes vs fp32. Max-rel error ~3e-3 against the fp64 reference
# (gate is 2e-2) — the fp32 accumulate in PSUM keeps the error sqrt-like.
MM_DT = mybir.dt.bfloat16

_PROGRAM = None
LAST_RESULTS = None  # test harness introspection (exec_time_ns when traced)


def _build_program():
    f32 = mybir.dt.float32
    nc = bacc.Bacc("TRN2", debug=False, num_devices=N_CORES)

    xt_d = nc.dram_tensor("xt", [D_IN, S], MM_DT, kind="ExternalInput").ap()
    wt_d = nc.dram_tensor("wt", [D_IN, D_OUT], MM_DT, kind="ExternalInput").ap()
    bb_d = nc.dram_tensor("bb", [128, D_OUT], f32, kind="ExternalInput").ap()
    out_d = nc.dram_tensor("out", [S, D_OUT], f32, kind="ExternalOutput").ap()

    # contraction dim on partitions, chunked by 128
    xt_v = xt_d.rearrange("(k p) s -> p k s", p=128)      # [128, KC, S]
    wt_v = wt_d.rearrange("(k p) o -> p k o", p=128)      # [128, KC, D_OUT]
    # output rows s = t*S_MACRO + u*128 + p
    out_v = out_d.rearrange(
        "(t u p) o -> t u p o", u=S_MACRO // 128, p=128
    )  # [T, 4, 128, D_OUT]

    with tile.TileContext(nc) as tc:
        with (
            tc.tile_pool(name="const", bufs=1) as cpool,
            tc.tile_pool(name="xin", bufs=3) as xpool,
            tc.tile_pool(name="outp", bufs=4) as opool,
            tc.tile_pool(name="psum", bufs=8, space="PSUM") as ppool,
        ):
            # PE HAM warmup: dummy matmuls with no DMA deps ramp the PE
            # clock (1.2 -> 2.4 GHz after ~3.4us of sustained activity,
            # free-running window) while the first input DMAs are still in
            # flight. memset on the vector engine: gpsimd took ~3.5us to
            # come up in the trace; DVE signals the PE within ~0.3us.
            warm_x = cpool.tile([128, 256], MM_DT)
            nc.vector.memset(warm_x[:], 0.0)
            warm_ps = ppool.tile([128, N_HALF], f32, tag="ps")
            for _ in range(16):
                nc.tensor.matmul(
                    warm_ps[:, :256], warm_x[:, :128], warm_x[:], start=True, stop=True
                )

            # Weight chunk 0 gates the first real matmul: split it into two
            # 128KB halves racing on both HWDGE rings (~120GB/s each early).
            # k=1..4 follow on the ACT ring; k=5..7 + bias go via the gpsimd
            # SWDGE ring (higher latency, but needed 10+us later anyway),
            # keeping the ACT ring clear for the chunks the PE needs first.
            wtile = cpool.tile([128, KC, D_OUT], MM_DT)
            nc.scalar.dma_start(wtile[:, 0, :N_HALF], wt_v[:, 0, :N_HALF])
            nc.sync.dma_start(wtile[:, 0, N_HALF:], wt_v[:, 0, N_HALF:])
            for k in range(1, 5):
                nc.scalar.dma_start(wtile[:, k, :], wt_v[:, k, :])
            for k in range(5, KC):
                nc.gpsimd.dma_start(wtile[:, k, :], wt_v[:, k, :])
            btile = cpool.tile([128, D_OUT], f32)
            nc.gpsimd.dma_start(btile[:], bb_d[:])

            NU = S_MACRO // 128
            NH = D_OUT // N_HALF
            for t in range(S // S_MACRO):
                xtile = xpool.tile([128, KC, S_MACRO], MM_DT)
                # split loads: matmuls on early k chunks start before the
                # later chunks arrive (finest split on the first tile,
                # which gates the pipeline ramp)
                s_sl = slice(t * S_MACRO, (t + 1) * S_MACRO)
                if t == 0:
                    # u0 of k0 split out so the very first matmul only waits
                    # on 32KB of x (plus the k0 weight half on the other ring)
                    nc.sync.dma_start(xtile[:, 0, :128], xt_v[:, 0, :128])
                    nc.sync.dma_start(xtile[:, 0, 128:], xt_v[:, 0, 128:S_MACRO])
                    for k in range(1, KC):
                        nc.sync.dma_start(xtile[:, k, :], xt_v[:, k, s_sl])
                else:
                    nc.sync.dma_start(
                        xtile[:, : KC // 2, :], xt_v[:, : KC // 2, s_sl]
                    )
                    nc.sync.dma_start(
                        xtile[:, KC // 2 :, :], xt_v[:, KC // 2 :, s_sl]
                    )
                if t == 0:
                    # ramp macro: k outermost with all 8 psum groups open —
                    # each arriving (x[k], W[k]) chunk pair feeds 8 matmuls
                    # (~1.8us PE work per ~1.9us of DMA), so the PE never
                    # idles long enough to re-throttle while the front-load
                    # streams in.
                    otiles, pss = [], []
                    for u in range(NU):
                        otile = opool.tile([128, D_OUT], f32)
                        otiles.append(otile)
                        for _h in range(NH):
                            ps = ppool.tile([128, N_HALF], f32, tag="ps")
                            pss.append(ps)
                    # h outermost within each k: the four h=0 matmuls only
                    # need the first 128KB weight half, buying ~1us for the
                    # h=1 half to land on the other ring.
                    for k in range(KC):
                        for h in range(NH):
                            for u in range(NU):
                                nc.tensor.matmul(
                                    pss[u * NH + h][:],
                                    xtile[:, k, u * 128 : (u + 1) * 128],
                                    wtile[:, k, h * N_HALF : (h + 1) * N_HALF],
                                    start=(k == 0),
                                    stop=(k == KC - 1),
                                )
                    for u in range(NU):
                        for h in range(NH):
                            nc.vector.tensor_add(
                                otiles[u][:, h * N_HALF : (h + 1) * N_HALF],
                                pss[u * NH + h][:],
                                btile[:, h * N_HALF : (h + 1) * N_HALF],
                            )
                        store_eng = nc.scalar if u % 2 == 0 else nc.sync
                        store_eng.dma_start(out_v[t, u], otiles[u][:])
                    continue
                last_t = t == S // S_MACRO - 1
                for u in range(NU):
                    otile = opool.tile([128, D_OUT], f32)
                    pss = []
                    for _h in range(NH):
                        ps = ppool.tile([128, N_HALF], f32, tag="ps")
                        pss.append(ps)
                    if last_t and u == NU - 1:
                        # very last tile: run the two halves as separate
                        # k-loops so h0's evac+store drains while h1's
                        # matmuls still run, leaving only 256KB behind the
                        # final matmul — split across both rings (2x128KB)
                        # to halve the exit-drain flush.
                        for h in range(NH):
                            for k in range(KC):
                                nc.tensor.matmul(
                                    pss[h][:],
                                    xtile[:, k, u * 128 : (u + 1) * 128],
                                    wtile[:, k, h * N_HALF : (h + 1) * N_HALF],
                                    start=(k == 0),
                                    stop=(k == KC - 1),
                                )
                            h_sl = slice(h * N_HALF, (h + 1) * N_HALF)
                            nc.vector.tensor_add(
                                otile[:, h_sl], pss[h][:], btile[:, h_sl]
                            )
                            if h == 0:
                                nc.scalar.dma_start(
                                    out_v[t, u][:, h_sl], otile[:, h_sl]
                                )
                            else:
                                q_sl = slice(2 * N_HALF // 2, 3 * N_HALF // 2)
                                r_sl = slice(3 * N_HALF // 2, 2 * N_HALF)
                                nc.sync.dma_start(
                                    out_v[t, u][:, q_sl], otile[:, q_sl]
                                )
                                nc.scalar.dma_start(
                                    out_v[t, u][:, r_sl], otile[:, r_sl]
                                )
                        continue
                    for k in range(KC):
                        # both output halves per k: consecutive matmuls
                        # share the stationary lhsT, halving LDW pressure
                        for h in range(NH):
                            nc.tensor.matmul(
                                pss[h][:],
                                xtile[:, k, u * 128 : (u + 1) * 128],  # lhsT [K,M]
                                wtile[:, k, h * N_HALF : (h + 1) * N_HALF],  # rhs [K,N]
                                start=(k == 0),
                                stop=(k == KC - 1),
                            )
                    for h in range(NH):
                        # evacuate psum with fused bias add
                        nc.vector.tensor_add(
                            otile[:, h * N_HALF : (h + 1) * N_HALF],
                            pss[h][:],
                            btile[:, h * N_HALF : (h + 1) * N_HALF],
                        )
                    if last_t:
                        # final macro: store halves on both rings as soon
                        # as each bias-add lands — halves the last flush
                        # the exit drain waits on
                        for h in range(NH):
                            eng = nc.scalar if h == 0 else nc.sync
                            eng.dma_start(
                                out_v[t, u][:, h * N_HALF : (h + 1) * N_HALF],
                                otile[:, h * N_HALF : (h + 1) * N_HALF],
                            )
                    else:
                        # alternate store rings to halve store-issue queuing
                        store_eng = nc.scalar if (t * 4 + u) % 2 == 0 else nc.sync
                        store_eng.dma_start(out_v[t, u], otile[:])

    nc.compile()
    return nc


def _get_program():
    global _PROGRAM
    if _PROGRAM is None:
        _PROGRAM = _build_program()
    return _PROGRAM


def kernel(x, task_ids, W, b, shared_A, shared_B, expert_A, expert_B, collab_w):
    global LAST_RESULTS
    x = np.asarray(x, dtype=np.float32)
    task_ids = np.asarray(task_ids)
    W = np.asarray(W, dtype=np.float32)
    b = np.asarray(b, dtype=np.float32)
    B = x.shape[0]
    assert B == N_CORES and x.shape[1:] == (S, D_IN)

    cw = np.float32(1.0 / (1.0 + np.exp(-np.float64(collab_w))))
    w_shared = (
        W
        + np.float32(cw * SCALING)
        * (np.asarray(shared_B, np.float32) @ np.asarray(shared_A, np.float32))
    ).astype(np.float32)
    ce = np.float32((1.0 - cw) * SCALING)

    np_in = mybir.dt.np(MM_DT)
    bb = np.ascontiguousarray(np.broadcast_to(b, (128, D_OUT)), dtype=np.float32)
    in_maps = []
    for bi in range(B):
        t = int(task_ids[bi])
        w_eff = w_shared + ce * (
            np.asarray(expert_B[t], np.float32) @ np.asarray(expert_A[t], np.float32)
        )
        in_maps.append(
            {
                "xt": np.ascontiguousarray(x[bi].T).astype(np_in),
                "wt": np.ascontiguousarray(w_eff.T).astype(np_in),
                "bb": bb,
            }
        )

    nc = _get_program()
    LAST_RESULTS = run_bass_kernel_spmd(nc, in_maps, list(range(N_CORES)))
    out = np.stack(
        [LAST_RESULTS.results[c]["out"] for c in range(N_CORES)], axis=0
    )
    return np.ascontiguousarray(out, dtype=np.float32)



# revision 9
# speedup vs baseline: 1.0533x; 1.0533x over previous
"""COLoRA linear kernel for 8 Trainium2 NeuronCores.

Reference computation (per batch element b with task t = task_ids[b]):

    out[b] = x[b] @ W.T + bias
           + cw      * 2 * (x[b] @ shared_A.T)    @ shared_B.T
           + (1-cw)  * 2 * (x[b] @ expert_A[t].T) @ expert_B[t].T
    cw = sigmoid(collab_w)

The rank-8 adapters fold exactly into the dense weight (associativity):

    W_eff[b] = W + cw*2*(shared_B @ shared_A) + (1-cw)*2*(expert_B[t] @ expert_A[t])
    out[b]   = x[b] @ x_eff[b].T + bias

so the device kernel is a single GEMM per core (PE-bound: 512 matmuls of
N=512 at ~216ns = 110.6us floor). Sharding is data-parallel over batch:
core c handles batch element c (B == n_cores == 8). The MoE routing
(task_ids gather) happens on the host at dispatch time.

x is pre-transposed on the host to [d_in, s] so the contraction dim lands
on SBUF partitions with contiguous DMA access. Operands are bf16 (single
pass through the PE at 1 cycle/column, FWL weight loads, half the input
DMA bytes); accumulation is fp32 in PSUM. Measured max-rel error vs the
fp64 reference is ~2.5e-3 (gate 2e-2).

Schedule notes (from perfetto/NTFF traces):
  - Framework preamble pins user code start at ~7us; HWDGE rings deliver
    first bytes ~1.7us after their engine issues the dma_start.
  - PE HAM clock gate: 1.2GHz until ~3.4us of sustained matmul activity;
    dummy warmups bridge the preamble->data window so real matmuls run
    at 2.4GHz almost immediately.
  - All early DMAs stay on the two HWDGE rings (sync/scalar). SWDGE
    (gpsimd) DMAs hold the 8 shared DMAHW completion lanes for ~2us past
    data-done and serialize later HWDGE issues (measured +6us regression).
  - Macro taper 512x7+256+128+128: the final macros' stores drain during
    the remaining compute, so the exit flush behind the last matmul is
    only 2x128KB on parallel rings instead of 512KB queued behind it.
"""

import os

import numpy as np

import concourse.bass as bass
import concourse.tile as tile
from concourse import bacc, mybir
from concourse.bass_utils import run_bass_kernel_spmd

try:  # tracing (BASS_TRACE) needs the axon NTFF hook; scrub if unavailable
    from antenv.axon_hooks import get_axon_ntff_profile_hook  # noqa: F401
except ImportError:
    os.environ.pop("BASS_TRACE", None)

N_CORES = 8
S = 4096        # rows per core (sequence length; one batch element per core)
D_IN = 1024
D_OUT = 1024
KC = D_IN // 128   # contraction chunks of 128
N_HALF = 512       # psum free dim (one bank)
SCALING = 2.0      # lora alpha/r = 16/8

# row-macro schedule: big macros amortize DMA; small tail macros let the
# output stores drain before the exit barrier waits on them
MACROS = [(r, 512) for r in range(0, 3584, 512)] + [(3584, 256), (3840, 128), (3968, 128)]

MM_DT = mybir.dt.bfloat16

_PROGRAM = None
LAST_RESULTS = None  # test harness introspection (exec_time_ns when traced)


def _build_program():
    f32 = mybir.dt.float32
    nc = bacc.Bacc("TRN2", debug=False, num_devices=N_CORES)

    xt_d = nc.dram_tensor("xt", [D_IN, S], MM_DT, kind="ExternalInput").ap()
    wt_d = nc.dram_tensor("wt", [D_IN, D_OUT], MM_DT, kind="ExternalInput").ap()
    bb_d = nc.dram_tensor("bb", [128, D_OUT], f32, kind="ExternalInput").ap()
    out_d = nc.dram_tensor("out", [S, D_OUT], f32, kind="ExternalOutput").ap()

    # contraction dim on partitions, chunked by 128
    xt_v = xt_d.rearrange("(k p) s -> p k s", p=128)      # [128, KC, S]
    wt_v = wt_d.rearrange("(k p) o -> p k o", p=128)      # [128, KC, D_OUT]
    out_r = out_d.rearrange("(r p) o -> r p o", p=128)    # [32, 128, D_OUT]

    NH = D_OUT // N_HALF
    n_macros = len(MACROS)

    with tile.TileContext(nc) as tc:
        with (
            tc.tile_pool(name="const", bufs=1) as cpool,
            tc.tile_pool(name="xin", bufs=3) as xpool,
            tc.tile_pool(name="outp", bufs=6) as opool,
            tc.tile_pool(name="psum", bufs=8, space="PSUM") as ppool,
        ):
            # PE HAM warmup: dummy matmuls with no DMA deps ramp the PE
            # clock (1.2 -> 2.4 GHz after ~3.4us of sustained activity,
            # free-running window) while the first input DMAs are in
            # flight. 16 x 256col bridges ~3.4us at the cold clock.
            warm_x = cpool.tile([128, 256], MM_DT)
            nc.vector.memset(warm_x[:], 0.0)
            warm_ps = ppool.tile([128, N_HALF], f32, tag="ps")
            for _ in range(16):
                nc.tensor.matmul(
                    warm_ps[:, :256], warm_x[:, :128], warm_x[:], start=True, stop=True
                )

            # Weight chunk 0 gates the first real matmul: its h=0 half
            # (128KB) leads the ACT ring while the h=1 half rides the sync
            # ring just behind the first x chunk. Remaining chunks + bias
            # stream on the ACT ring (~200GB/s clean), staying ahead of
            # the warm PE's one-chunk-per-1.7us consumption.
            wtile = cpool.tile([128, KC, D_OUT], MM_DT)
            nc.scalar.dma_start(wtile[:, 0, :N_HALF], wt_v[:, 0, :N_HALF])
            for k in range(1, KC):
                nc.scalar.dma_start(wtile[:, k, :], wt_v[:, k, :])
            btile = cpool.tile([128, D_OUT], f32)
            nc.scalar.dma_start(btile[:], bb_d[:])

            store_ct = 0  # alternates steady-state stores across rings
            for mi, (r0, nr) in enumerate(MACROS):
                nu = nr // 128
                s_sl = slice(r0, r0 + nr)
                xtile = xpool.tile([128, KC, nr], MM_DT)
                if mi == 0:
                    # u0 of k0 split out so the very first matmul only
                    # waits on 32KB of x plus the k0 weight half; the k0
                    # h=1 weight half follows on this ring before the
                    # remaining k chunks.
                    nc.sync.dma_start(xtile[:, 0, :128], xt_v[:, 0, :128])
                    nc.sync.dma_start(xtile[:, 0, 128:], xt_v[:, 0, 128:nr])
                    nc.sync.dma_start(wtile[:, 0, N_HALF:], wt_v[:, 0, N_HALF:])
                    for k in range(1, KC):
                        nc.sync.dma_start(xtile[:, k, :], xt_v[:, k, s_sl])
                elif nr > 128:
                    nc.sync.dma_start(xtile[:, : KC // 2, :], xt_v[:, : KC // 2, s_sl])
                    nc.sync.dma_start(xtile[:, KC // 2 :, :], xt_v[:, KC // 2 :, s_sl])
                else:
                    nc.sync.dma_start(xtile[:, :, :], xt_v[:, :, s_sl])

                if mi == 0:
                    # ramp macro: k outermost with all 8 psum groups open —
                    # each arriving (x[k], W[k]) chunk pair feeds 8 matmuls,
                    # so the PE stays busy while the front-load streams in.
                    # h outermost within k: the four h=0 matmuls need only
                    # the 128KB weight half that leads the ACT ring.
                    otiles, pss = [], []
                    for u in range(nu):
                        otiles.append(opool.tile([128, D_OUT], f32, name="otile"))
                        for _h in range(NH):
                            pss.append(ppool.tile([128, N_HALF], f32, tag="ps", name="ps"))
                    for k in range(KC):
                        for h in range(NH):
                            for u in range(nu):
                                nc.tensor.matmul(
                                    pss[u * NH + h][:],
                                    xtile[:, k, u * 128 : (u + 1) * 128],
                                    wtile[:, k, h * N_HALF : (h + 1) * N_HALF],
                                    start=(k == 0),
                                    stop=(k == KC - 1),
                                )
                    for u in range(nu):
                        for h in range(NH):
                            nc.vector.tensor_add(
                                otiles[u][:, h * N_HALF : (h + 1) * N_HALF],
                                pss[u * NH + h][:],
                                btile[:, h * N_HALF : (h + 1) * N_HALF],
                            )
                        store_eng = nc.scalar if u % 2 == 0 else nc.sync
                        store_eng.dma_start(out_r[r0 // 128 + u], otiles[u][:])
                        store_ct += 1
                    continue

                final_two = mi >= n_macros - 2  # the 128-row tail macros
                for u in range(nu):
                    rb = r0 // 128 + u
                    otile = opool.tile([128, D_OUT], f32)
                    pss = [
                        ppool.tile([128, N_HALF], f32, tag="ps", name="ps")
                        for _ in range(NH)
                    ]
                    if final_two:
                        # tail macros: separate k-loops per output half so
                        # h0's evac+store drains while h1's matmuls run;
                        # the very last 256KB flush splits across both
                        # rings (2x128KB in parallel).
                        for h in range(NH):
                            for k in range(KC):
                                nc.tensor.matmul(
                                    pss[h][:],
                                    xtile[:, k, u * 128 : (u + 1) * 128],
                                    wtile[:, k, h * N_HALF : (h + 1) * N_HALF],
                                    start=(k == 0),
                                    stop=(k == KC - 1),
                                )
                            h_sl = slice(h * N_HALF, (h + 1) * N_HALF)
                            nc.vector.tensor_add(
                                otile[:, h_sl], pss[h][:], btile[:, h_sl]
                            )
                            if mi == n_macros - 1 and h == NH - 1:
                                q_sl = slice(N_HALF, N_HALF + N_HALF // 2)
                                r_sl = slice(N_HALF + N_HALF // 2, D_OUT)
                                nc.sync.dma_start(out_r[rb][:, q_sl], otile[:, q_sl])
                                nc.scalar.dma_start(out_r[rb][:, r_sl], otile[:, r_sl])
                            else:
                                eng = nc.scalar if h == 0 else nc.sync
                                eng.dma_start(out_r[rb][:, h_sl], otile[:, h_sl])
                        continue
                    for k in range(KC):
                        # both output halves per k: consecutive matmuls
                        # share the stationary lhsT, halving LDW pressure
                        for h in range(NH):
                            nc.tensor.matmul(
                                pss[h][:],
                                xtile[:, k, u * 128 : (u + 1) * 128],  # lhsT [K,M]
                                wtile[:, k, h * N_HALF : (h + 1) * N_HALF],  # rhs
                                start=(k == 0),
                                stop=(k == KC - 1),
                            )
                    for h in range(NH):
                        # evacuate psum with fused bias add
                        nc.vector.tensor_add(
                            otile[:, h * N_HALF : (h + 1) * N_HALF],
                            pss[h][:],
                            btile[:, h * N_HALF : (h + 1) * N_HALF],
                        )
                    if mi == n_macros - 3:
                        # 256-row macro: store halves on both rings as soon
                        # as each bias-add lands, draining ahead of the tail
                        for h in range(NH):
                            eng = (
                                nc.scalar if (u + h) % 2 == 0 else nc.sync
                            )
                            eng.dma_start(
                                out_r[rb][:, h * N_HALF : (h + 1) * N_HALF],
                                otile[:, h * N_HALF : (h + 1) * N_HALF],
                            )
                    else:
                        # alternate store rings to halve store-issue queuing
                        store_eng = nc.scalar if store_ct % 2 == 0 else nc.sync
                        store_eng.dma_start(out_r[rb], otile[:])
                        store_ct += 1

    nc.compile()
    return nc


def _get_program():
    global _PROGRAM
    if _PROGRAM is None:
        _PROGRAM = _build_program()
    return _PROGRAM


def kernel(x, task_ids, W, b, shared_A, shared_B, expert_A, expert_B, collab_w):
    global LAST_RESULTS
    x = np.asarray(x, dtype=np.float32)
    task_ids = np.asarray(task_ids)
    W = np.asarray(W, dtype=np.float32)
    b = np.asarray(b, dtype=np.float32)
    B = x.shape[0]
    assert B == N_CORES and x.shape[1:] == (S, D_IN)

    cw = np.float32(1.0 / (1.0 + np.exp(-np.float64(collab_w))))
    w_shared = (
        W
        + np.float32(cw * SCALING)
        * (np.asarray(shared_B, np.float32) @ np.asarray(shared_A, np.float32))
    ).astype(np.float32)
    ce = np.float32((1.0 - cw) * SCALING)

    np_in = mybir.dt.np(MM_DT)
    bb = np.ascontiguousarray(np.broadcast_to(b, (128, D_OUT)), dtype=np.float32)
    in_maps = []
    for bi in range(B):
        t = int(task_ids[bi])
        w_eff = w_shared + ce * (
            np.asarray(expert_B[t], np.float32) @ np.asarray(expert_A[t], np.float32)
        )
        in_maps.append(
            {
                "xt": np.ascontiguousarray(x[bi].T).astype(np_in),
                "wt": np.ascontiguousarray(w_eff.T).astype(np_in),
                "bb": bb,
            }
        )

    nc = _get_program()
    LAST_RESULTS = run_bass_kernel_spmd(nc, in_maps, list(range(N_CORES)))
    out = np.stack(
        [LAST_RESULTS.results[c]["out"] for c in range(N_CORES)], axis=0
    )
    return np.ascontiguousarray(out, dtype=np.float32)
